# revision 3
# baseline (speedup 1.0000x reference)
"""Trainium2 Bass kernel for nn_Downsample2d: depthwise 4x4 'linear' anti-alias
blur (k = [1,3,3,1]/8 separable), stride 2, reflect padding 1.

Input  x [8, 128, 256, 256] f32  ->  Output [8, 128, 128, 128] f32.

Strategy (pure data parallel over the 1024 (n, c) planes, 128 per core):
  - SBUF layout: image ROWS on the 128 partitions, (plane, col) on the free dim.
  - Vertical blur + 2x downsample as TensorE matmuls: V = Wv.T @ X where
    Wv [256, 128] is a constant band matrix with the reflect padding and the
    full 1/64 scale folded in. K=256 is split into two K=128 matmuls
    (rows 0-127 / 128-255) accumulated in PSUM.
  - PSUM -> SBUF copy on ScalarE.
  - Horizontal blur + 2x downsample as a 3-op VectorE stencil on the
    half-height data (P = V[2j]+V[2j+1], Q = V[2j-1]+V[2j+2], out = 3P + Q)
    plus tiny edge-column fixups.
"""
import numpy as np

N, C, H, W = 8, 128, 256, 256
HO, WO = H // 2, W // 2
N_CORES = 8
PLANES = N * C                    # 1024
P_CORE = PLANES // N_CORES        # 128 planes per core

_K1 = np.array([1.0, 3.0, 3.0, 1.0])


def make_wv(h=H):
    """Vertical blur+downsample band matrix [h, h//2]; reflect + 1/64 folded in."""
    wv = np.zeros((h, h // 2), dtype=np.float64)
    for i in range(h // 2):
        for a in range(4):
            r = 2 * i - 1 + a
            if r < 0:
                r = -r
            if r >= h:
                r = 2 * h - 2 - r
            wv[r, i] += _K1[a] / 64.0
    return wv.astype(np.float32)


def build_program(p_core=P_CORE, g=16, use_f32r=True, enable_asserts=False):
    """Build and compile the per-core Bass program.

    p_core: planes handled by one core; g: planes per pipeline group.
    """
    import concourse.bacc as bacc
    import concourse.tile as tile
    from concourse import mybir

    assert p_core % g == 0 and g % 2 == 0
    f32 = mybir.dt.float32
    # float32r has the same bit layout as float32; declaring the input tensors
    # and tiles as f32r end-to-end satisfies the BIR verifier's "rounded to
    # FP32r" producer check while the host still supplies np.float32 data.
    mmdt = mybir.dt.float32r if use_f32r else f32
    mult, add = mybir.AluOpType.mult, mybir.AluOpType.add

    nc = bacc.Bacc(
        "TRN2",
        target_bir_lowering=False,
        debug=False,
        enable_asserts=enable_asserts,
        num_devices=N_CORES,
    )
    x = nc.dram_tensor("x", [p_core, H, W], mmdt, kind="ExternalInput")
    wv = nc.dram_tensor("wv", [H, HO], mmdt, kind="ExternalInput")
    y = nc.dram_tensor("y", [p_core, HO, WO], f32, kind="ExternalOutput")

    xr = x.ap().rearrange("p h w -> h p w")   # [256, p_core, 256]
    yr = y.ap().rearrange("p h w -> h p w")   # [128, p_core, 128]

    with tile.TileContext(nc) as tc:
        with (
            tc.tile_pool(name="wpool", bufs=1) as wpool,
            tc.tile_pool(name="xpool", bufs=2) as xpool,
            tc.tile_pool(name="vpool", bufs=2) as vpool,
            tc.tile_pool(name="opool", bufs=2) as opool,
            tc.tile_pool(name="tpool", bufs=2) as tpool,
            tc.tile_pool(name="psum", bufs=4, space="PSUM") as psum,
        ):
            wvt = wpool.tile([128, HO], mmdt, tag="wvt")
            wvb = wpool.tile([128, HO], mmdt, tag="wvb")
            nc.sync.dma_start(wvt[:], wv[0:128, :])
            nc.sync.dma_start(wvb[:], wv[128:256, :])

            for gi in range(p_core // g):
                g0 = gi * g
                xt = xpool.tile([128, g, W], mmdt, tag="xt")
                xb = xpool.tile([128, g, W], mmdt, tag="xb")
                nc.sync.dma_start(xt[:], xr[0:128, g0:g0 + g, :])
                nc.sync.dma_start(xb[:], xr[128:256, g0:g0 + g, :])

                vs = vpool.tile([128, g, W], f32, tag="vs")
                for s in range(g // 2):
                    vp = psum.tile([128, 2, W], f32, tag="vp")
                    nc.tensor.matmul(
                        vp[:],
                        wvt[:],
                        xt[:, 2 * s:2 * s + 2, :],
                        start=True, stop=False,
                    )
                    nc.tensor.matmul(
                        vp[:],
                        wvb[:],
                        xb[:, 2 * s:2 * s + 2, :],
                        start=False, stop=True,
                    )
                    nc.scalar.copy(vs[:, 2 * s:2 * s + 2, :], vp[:])

                # horizontal stencil: out = 3*(V[2j]+V[2j+1]) + (V[2j-1]+V[2j+2])
                pt = tpool.tile([128, g, WO], f32, tag="pt")
                qt = tpool.tile([128, g, WO - 2], f32, tag="qt")
                ot = opool.tile([128, g, WO], f32, tag="ot")
                nc.vector.tensor_add(pt[:], vs[:, :, 0:W:2], vs[:, :, 1:W:2])
                nc.vector.tensor_add(qt[:], vs[:, :, 1:W - 3:2], vs[:, :, 4:W:2])
                nc.vector.scalar_tensor_tensor(
                    ot[:, :, 1:WO - 1], pt[:, :, 1:WO - 1], 3.0, qt[:], mult, add
                )
                # edge columns: j=0 -> 3V0+4V1+V2 ; j=WO-1 -> 3V[W-1]+4V[W-2]+V[W-3]
                e0 = tpool.tile([128, g, 1], f32, tag="e0")
                e1 = tpool.tile([128, g, 1], f32, tag="e1")
                nc.vector.scalar_tensor_tensor(
                    e0[:], vs[:, :, 1:2], 4.0, vs[:, :, 2:3], mult, add
                )
                nc.vector.scalar_tensor_tensor(
                    ot[:, :, 0:1], vs[:, :, 0:1], 3.0, e0[:], mult, add
                )
                nc.vector.scalar_tensor_tensor(
                    e1[:], vs[:, :, W - 2:W - 1], 4.0, vs[:, :, W - 3:W - 2], mult, add
                )
                nc.vector.scalar_tensor_tensor(
                    ot[:, :, WO - 1:WO], vs[:, :, W - 1:W], 3.0, e1[:], mult, add
                )

                nc.sync.dma_start(yr[:, g0:g0 + g, :], ot[:])

    nc.compile()
    return nc


_CACHE = {}


def _get_program():
    key = "prog"
    if key not in _CACHE:
        _CACHE[key] = build_program()
    return _CACHE[key]


def kernel(x):
    from concourse.bass_utils import run_bass_kernel_spmd

    x = np.asarray(x, dtype=np.float32)
    assert x.shape == (N, C, H, W), x.shape
    xf = np.ascontiguousarray(x).reshape(PLANES, H, W)
    wv_np = make_wv()

    nc = _get_program()
    in_maps = [
        {"x": xf[k * P_CORE:(k + 1) * P_CORE], "wv": wv_np} for k in range(N_CORES)
    ]
    res = run_bass_kernel_spmd(nc, in_maps, core_ids=list(range(N_CORES)))
    y = np.concatenate([res.results[k]["y"] for k in range(N_CORES)], axis=0)
    return y.reshape(N, C, HO, WO)


# revision 5
# speedup vs baseline: 1.1672x; 1.1672x over previous
"""Trainium2 Bass kernel for nn_Downsample2d: depthwise 4x4 'linear' anti-alias
blur (k = [1,3,3,1]/8 separable), stride 2, reflect padding 1.

Input  x [8, 128, 256, 256] f32  ->  Output [8, 128, 128, 128] f32.

Strategy (pure data parallel over the 1024 (n, c) planes, 128 per core):
  - SBUF layout: image ROWS on the 128 partitions, (plane, col) on the free dim.
  - Vertical blur + 2x downsample as TensorE matmuls: V = Wv.T @ X where
    Wv [256, 128] is a constant band matrix with the reflect padding and the
    full 1/64 scale folded in. K=256 is split into two K=128 matmuls
    (rows 0-127 / 128-255) accumulated in PSUM.
  - PSUM -> SBUF copy on ScalarE.
  - Horizontal blur + 2x downsample as a 3-op VectorE stencil on the
    half-height data (P = V[2j]+V[2j+1], Q = V[2j-1]+V[2j+2], out = 3P + Q)
    plus tiny edge-column fixups.
"""
import numpy as np

N, C, H, W = 8, 128, 256, 256
HO, WO = H // 2, W // 2
N_CORES = 8
PLANES = N * C                    # 1024
P_CORE = PLANES // N_CORES        # 128 planes per core

_K1 = np.array([1.0, 3.0, 3.0, 1.0])


def make_wv(h=H):
    """Vertical blur+downsample band matrix [h, h//2]; reflect + 1/64 folded in."""
    wv = np.zeros((h, h // 2), dtype=np.float64)
    for i in range(h // 2):
        for a in range(4):
            r = 2 * i - 1 + a
            if r < 0:
                r = -r
            if r >= h:
                r = 2 * h - 2 - r
            wv[r, i] += _K1[a] / 64.0
    return wv.astype(np.float32)


def build_program(p_core=P_CORE, g=8, use_f32r=True, enable_asserts=False):
    """Build and compile the per-core Bass program.

    p_core: planes handled by one core; g: planes per pipeline group.

    SBUF input layout packs row pairs per partition: partition p holds input
    rows {2p, 2p+1} of each plane in the group, so the load DMA reads 2 KiB
    contiguous chunks from DRAM. The vertical matmul splits into an even-rows
    and an odd-rows weight matrix (Wv[0::2], Wv[1::2]) accumulated in PSUM.
    """
    import concourse.bacc as bacc
    import concourse.tile as tile
    from concourse import mybir

    assert p_core % g == 0 and g % 2 == 0
    f32 = mybir.dt.float32
    # float32r has the same bit layout as float32; declaring the input tensors
    # and tiles as f32r end-to-end satisfies the BIR verifier's "rounded to
    # FP32r" producer check while the host still supplies np.float32 data.
    mmdt = mybir.dt.float32r if use_f32r else f32
    mult, add = mybir.AluOpType.mult, mybir.AluOpType.add

    nc = bacc.Bacc(
        "TRN2",
        target_bir_lowering=False,
        debug=False,
        enable_asserts=enable_asserts,
        num_devices=N_CORES,
    )
    x = nc.dram_tensor("x", [p_core, H, W], mmdt, kind="ExternalInput")
    wv = nc.dram_tensor("wv", [H, HO], mmdt, kind="ExternalInput")
    y = nc.dram_tensor("y", [p_core, HO, WO], f32, kind="ExternalOutput")

    # partition p <- rows {2p, 2p+1}; free = (plane, row-pair (2) x col)
    xr = x.ap().rearrange("n (h tw) w -> h n (tw w)", tw=2)  # [128, p_core, 512]
    yr = y.ap().rearrange("p h w -> h p w")                  # [128, p_core, 128]

    with tile.TileContext(nc) as tc:
        with (
            tc.tile_pool(name="wpool", bufs=1) as wpool,
            tc.tile_pool(name="xpool", bufs=3) as xpool,
            tc.tile_pool(name="vpool", bufs=2) as vpool,
            tc.tile_pool(name="opool", bufs=2) as opool,
            tc.tile_pool(name="tpool", bufs=2) as tpool,
            tc.tile_pool(name="psum", bufs=4, space="PSUM") as psum,
        ):
            # we = Wv[0::2] (even input rows), wo = Wv[1::2] (odd input rows)
            we = wpool.tile([128, HO], mmdt, tag="we")
            wo = wpool.tile([128, HO], mmdt, tag="wo")
            nc.sync.dma_start(we[:], wv[0:256:2, :])
            nc.sync.dma_start(wo[:], wv[1:256:2, :])

            for gi in range(p_core // g):
                g0 = gi * g
                xt = xpool.tile([128, g, 2 * W], mmdt, tag="xt")
                nc.sync.dma_start(xt[:], xr[:, g0:g0 + g, :])

                vs = vpool.tile([128, g, W], f32, tag="vs")
                for s in range(g // 2):
                    vp = psum.tile([128, 2, W], f32, tag="vp")
                    nc.tensor.matmul(
                        vp[:],
                        we[:],
                        xt[:, 2 * s:2 * s + 2, 0:W],
                        start=True, stop=False,
                    )
                    nc.tensor.matmul(
                        vp[:],
                        wo[:],
                        xt[:, 2 * s:2 * s + 2, W:2 * W],
                        start=False, stop=True,
                    )
                    nc.scalar.copy(vs[:, 2 * s:2 * s + 2, :], vp[:])

                # horizontal stencil: out = 3*(V[2j]+V[2j+1]) + (V[2j-1]+V[2j+2])
                pt = tpool.tile([128, g, WO], f32, tag="pt")
                qt = tpool.tile([128, g, WO - 2], f32, tag="qt")
                ot = opool.tile([128, g, WO], f32, tag="ot")
                nc.vector.tensor_add(pt[:], vs[:, :, 0:W:2], vs[:, :, 1:W:2])
                nc.vector.tensor_add(qt[:], vs[:, :, 1:W - 3:2], vs[:, :, 4:W:2])
                nc.vector.scalar_tensor_tensor(
                    ot[:, :, 1:WO - 1], pt[:, :, 1:WO - 1], 3.0, qt[:], mult, add
                )
                # edge columns: j=0 -> 3V0+4V1+V2 ; j=WO-1 -> 3V[W-1]+4V[W-2]+V[W-3]
                e0 = tpool.tile([128, g, 1], f32, tag="e0")
                e1 = tpool.tile([128, g, 1], f32, tag="e1")
                nc.vector.scalar_tensor_tensor(
                    e0[:], vs[:, :, 1:2], 4.0, vs[:, :, 2:3], mult, add
                )
                nc.vector.scalar_tensor_tensor(
                    ot[:, :, 0:1], vs[:, :, 0:1], 3.0, e0[:], mult, add
                )
                nc.vector.scalar_tensor_tensor(
                    e1[:], vs[:, :, W - 2:W - 1], 4.0, vs[:, :, W - 3:W - 2], mult, add
                )
                nc.vector.scalar_tensor_tensor(
                    ot[:, :, WO - 1:WO], vs[:, :, W - 1:W], 3.0, e1[:], mult, add
                )

                # store on SWDGE (gpsimd) so it doesn't serialize with the
                # input loads on the Sync HWDGE ring
                nc.gpsimd.dma_start(yr[:, g0:g0 + g, :], ot[:])

    nc.compile()
    return nc


_CACHE = {}


def _get_program():
    key = "prog"
    if key not in _CACHE:
        _CACHE[key] = build_program()
    return _CACHE[key]


def kernel(x):
    from concourse.bass_utils import run_bass_kernel_spmd

    x = np.asarray(x, dtype=np.float32)
    assert x.shape == (N, C, H, W), x.shape
    xf = np.ascontiguousarray(x).reshape(PLANES, H, W)
    wv_np = make_wv()

    nc = _get_program()
    in_maps = [
        {"x": xf[k * P_CORE:(k + 1) * P_CORE], "wv": wv_np} for k in range(N_CORES)
    ]
    res = run_bass_kernel_spmd(nc, in_maps, core_ids=list(range(N_CORES)))
    y = np.concatenate([res.results[k]["y"] for k in range(N_CORES)], axis=0)
    return y.reshape(N, C, HO, WO)
